# revision 12
# baseline (speedup 1.0000x reference)
"""Causal multi-head attention (B=2, T=2048, C=1024, H=16, D=64) on 8 TRN2 cores.

Sharding: core c -> batch b = c//4, head-group hg = c%4 (4 heads/core).
Each core computes its 4 heads' attention and a partial output projection
(contraction over its 256 feature columns of W_proj); the host sums the 4
partials per batch.

All device-side layouts are transposed on host so the kernel needs no
on-device transposes:
  xT  [C, T]   = x[b].T
  wqT/wkT/wvT [C, 256] = W_{q,k,v}[rows].T
  wpT [256, C] = W_proj[:, rows].T
Attention math per head (D=64):
  qT/kT [64, T] = (wqT chunk).T @ xT            (PE, f32r)
  v     [T, 64] = (xT chunk).T @ wvT            (+ ones col -> row sums)
  sT    [s, t]  = kT.T @ qT                     (K=64)
  pT            = exp(sT/8)  (no max-subtraction needed: |scores/8| < ~7)
  causal: tri-mask on diagonal 128-blocks only; lower kb blocks restricted
  oT_aug[65, t] = v_aug.T @ pT    (row 64 = softmax sums, free)
  oT_norm       = oT * (1/sums)   (reciprocal + DMA partition-broadcast + mul)
  y     [t, p]  = oT.T @ wpT      (partial over this core's 256 features)
"""

import sys

sys.path.insert(0, "/opt/trn_rl_repo")

import numpy as np

import concourse.bass as bass  # noqa: E402
import concourse.mybir as mybir  # noqa: E402
import concourse.tile as tile  # noqa: E402
from concourse import bacc  # noqa: E402
from concourse.bass_utils import run_bass_kernel_spmd  # noqa: E402

F32 = mybir.dt.float32
F32R = mybir.dt.float32r

T = 2048
C = 1024
HL = 4  # heads per core
D = 64
HD = HL * D  # 256 local feature dim
TC = 512  # t-chunk for attention
NTC = T // TC  # 4
SB = 128  # s block
NSB = T // SB  # 16
N_CORES = 8

# matmul input dtype: float32r streams 1 row/cycle at N>=256 (4x faster than
# plain float32) with near-fp32 accuracy (fp32 accumulate).
MM_DT = F32R
BF16 = mybir.dt.bfloat16
# attention (scores/PV) operand dtype
ATT_DT = BF16


def _mm(ap):
    return ap


def _build_program():
    nc = bacc.Bacc("TRN2", target_bir_lowering=False, debug=False)

    xT_d = nc.dram_tensor("xT", [C, T], MM_DT, kind="ExternalInput")
    wqT_d = nc.dram_tensor("wqT", [C, HD], MM_DT, kind="ExternalInput")
    wkT_d = nc.dram_tensor("wkT", [C, HD], MM_DT, kind="ExternalInput")
    wvT_d = nc.dram_tensor("wvT", [C, HD], MM_DT, kind="ExternalInput")
    wpT_d = nc.dram_tensor("wpT", [HD, C], MM_DT, kind="ExternalInput")
    mask_d = nc.dram_tensor("mask", [SB, SB], ATT_DT, kind="ExternalInput")
    ones_d = nc.dram_tensor("ones", [SB, 1], ATT_DT, kind="ExternalInput")
    y_d = nc.dram_tensor("y", [T, C], F32, kind="ExternalOutput")

    NKC = C // SB  # 8 contraction chunks of 128

    with tile.TileContext(nc) as tc:
        with (
            tc.tile_pool(name="persist", bufs=1) as persist,
            tc.tile_pool(name="pt", bufs=10) as pt_pool,
            tc.tile_pool(name="ysb", bufs=3) as ysb_pool,
            tc.tile_pool(name="xtj", bufs=2) as xtj_pool,
            tc.tile_pool(name="norm", bufs=2) as norm_pool,
            tc.tile_pool(name="ps_s", bufs=4, space="PSUM") as ps_s,
            tc.tile_pool(name="ps_o", bufs=4, space="PSUM") as ps_o,
        ):
            ps_mm = ps_s
            ps_y = ps_s
            # ---- persistent SBUF tiles (packed [128, nchunks*width]) ----
            wq_sb = persist.tile([SB, NKC * HD], MM_DT)
            wk_sb = persist.tile([SB, NKC * HD], MM_DT)
            wv_sb = persist.tile([SB, NKC * HD], MM_DT)
            wp_sb = persist.tile([SB, (HD // SB) * C], MM_DT)  # 2 chunks of [128, 1024]
            qT_sb = persist.tile([SB, 2 * T], ATT_DT)  # grp g: heads 2g,2g+1
            kT_sb = persist.tile([SB, 2 * T], ATT_DT)
            v_sb = persist.tile([SB, NSB * HL * (D + 1)], ATT_DT)  # s-chunk n: n*260
            oT_sb = persist.tile([SB, 2 * T], MM_DT)
            mask_sb = persist.tile([SB, SB], ATT_DT)

            def load_packed(sb, dram_ap, width):
                n = dram_ap.shape[0] // SB
                nc.sync.dma_start(
                    sb[:].rearrange("p (n w) -> p n w", n=n),
                    dram_ap.rearrange("(n p) w -> p n w", p=SB),
                )

            load_packed(wq_sb, wqT_d.ap(), HD)
            load_packed(wk_sb, wkT_d.ap(), HD)
            load_packed(wv_sb, wvT_d.ap(), HD)
            load_packed(wp_sb, wpT_d.ap(), C)
            nc.sync.dma_start(mask_sb[:], mask_d.ap())

            # ones columns of v_aug (col 64 of each head's 65-col group)
            v_ones = v_sb[:].rearrange("p (k d) -> p k d", k=NSB * HL)[:, :, D : D + 1]
            ones_src = ones_d.ap().unsqueeze(1).to_broadcast((SB, NSB * HL, 1))
            nc.sync.dma_start(v_ones, ones_src)

            # ---- phase 2: QKV projections (xT streamed per t-slice j) ----
            xT_packed = xT_d.ap().rearrange("(n p) t -> p n t", p=SB)
            for j in range(NTC):
                xTj = xtj_pool.tile([SB, NKC * TC], MM_DT, tag="xtj", name=f"xtj_{j}")
                nc.sync.dma_start(
                    xTj[:].rearrange("p (n w) -> p n w", n=NKC),
                    xT_packed[:, :, j * TC : (j + 1) * TC],
                )
                for dst, w_sb, eng in ((qT_sb, wq_sb, "s"), (kT_sb, wk_sb, "v")):
                    for g in range(2):  # partition group (2 heads each)
                        ps = ps_mm.tile([SB, TC], F32, tag="s")
                        for n in range(NKC):
                            nc.tensor.matmul(
                                ps[:],
                                _mm(w_sb[:, n * HD + g * SB : n * HD + (g + 1) * SB]),
                                _mm(xTj[:, n * TC : (n + 1) * TC]),
                                start=(n == 0),
                                stop=(n == NKC - 1),
                            )
                        dst_ap = dst[:, g * T + j * TC : g * T + (j + 1) * TC]
                        if eng == "s":
                            nc.scalar.copy(dst_ap, ps[:])
                        else:
                            nc.vector.tensor_copy(dst_ap, ps[:])

                for n in range(HL * j, HL * j + HL):  # v: out [128 s, 256 d]
                    off = (n - HL * j) * SB
                    ps = ps_mm.tile([SB, TC], F32, tag="s")
                    for m in range(NKC):
                        nc.tensor.matmul(
                            ps[:, 0:HD],
                            _mm(xTj[:, m * TC + off : m * TC + off + SB]),
                            _mm(wv_sb[:, m * HD : (m + 1) * HD]),
                            start=(m == 0),
                            stop=(m == NKC - 1),
                        )
                    dst = v_sb[:, n * HL * (D + 1) : (n + 1) * HL * (D + 1)].rearrange(
                        "p (h d) -> p h d", h=HL
                    )[:, :, 0:D]
                    src = ps[:, 0:HD].rearrange("p (h d) -> p h d", h=HL)
                    nc.scalar.copy(dst, src)

            # ---- phase 3: attention, 4 heads interleaved per (j, kb) ----
            # scores for head pairs run concurrently in disjoint PE row groups
            for j in range(NTC):
                po = [
                    ps_o.tile([D + 1, TC], F32, tag="o", name=f"po_{j}_{h}")
                    for h in range(HL)
                ]
                last_kb = HL * j + 3
                for kb in range(last_kb + 1):
                    tstart = max(0, (kb - HL * j) * SB)
                    pss = [
                        ps_s.tile([SB, TC], F32, tag="s", name=f"pss_{j}_{kb}_{h}")
                        for h in range(HL)
                    ]
                    pT = [
                        pt_pool.tile([SB, TC], ATT_DT, tag="pt", name=f"pt_{j}_{kb}_{h}")
                        for h in range(HL)
                    ]
                    for h in range(HL):
                        hp = D * (h % 2)
                        hg = h // 2
                        nc.tensor.matmul(
                            pss[h][:, tstart:],
                            _mm(
                                kT_sb[
                                    hp : hp + D,
                                    hg * T + kb * SB : hg * T + (kb + 1) * SB,
                                ]
                            ),
                            _mm(
                                qT_sb[
                                    hp : hp + D,
                                    hg * T + j * TC + tstart : hg * T + (j + 1) * TC,
                                ]
                            ),
                            start=True,
                            stop=True,
                            tile_position=(hp, 0),
                        )
                    for h in range(HL):
                        nc.scalar.activation(
                            pT[h][:, tstart:],
                            pss[h][:, tstart:],
                            mybir.ActivationFunctionType.Exp,
                            scale=float(D) ** -0.5,
                        )
                        if kb >= HL * j:  # diagonal block: causal tri-mask
                            nc.vector.tensor_mul(
                                pT[h][:, tstart : tstart + SB],
                                pT[h][:, tstart : tstart + SB],
                                mask_sb[:],
                            )
                    for h in range(HL):
                        nc.tensor.matmul(
                            po[h][:, tstart:],
                            _mm(
                                v_sb[
                                    :,
                                    kb * HL * (D + 1)
                                    + h * (D + 1) : kb * HL * (D + 1)
                                    + (h + 1) * (D + 1),
                                ]
                            ),
                            _mm(pT[h][:, tstart:]),
                            start=(kb == 0),
                            stop=(kb == last_kb),
                        )
                # normalize: oT_sb[...] = po[0:64] * (1 / po[64])
                for h in range(HL):
                    hp = D * (h % 2)
                    hg = h // 2
                    row = norm_pool.tile([1, TC], F32, tag="row", name=f"row_{j}_{h}")
                    nc.vector.reciprocal(row[:], po[h][D : D + 1, :])
                    bc = norm_pool.tile([D, TC], F32, tag="bc", name=f"bc_{j}_{h}")
                    nc.gpsimd.partition_broadcast(bc[:], row[:])
                    nc.vector.tensor_mul(
                        oT_sb[hp : hp + D, hg * T + j * TC : hg * T + (j + 1) * TC],
                        po[h][0:D, :],
                        bc[:],
                    )

            # ---- phase 4: output projection ----
            for i in range(NSB):
                for half in range(2):
                    ps = ps_y.tile([SB, TC], F32, tag="s")
                    for g in range(2):
                        nc.tensor.matmul(
                            ps[:],
                            _mm(oT_sb[:, g * T + i * SB : g * T + (i + 1) * SB]),
                            _mm(wp_sb[:, g * C + half * TC : g * C + (half + 1) * TC]),
                            start=(g == 0),
                            stop=(g == 1),
                        )
                    y_sb = ysb_pool.tile([SB, TC], F32, tag="ysb")
                    nc.vector.tensor_copy(y_sb[:], ps[:])
                    nc.sync.dma_start(
                        y_d.ap()[i * SB : (i + 1) * SB, half * TC : (half + 1) * TC],
                        y_sb[:],
                    )

    nc.compile()
    return nc


_NC_CACHE = None


def _get_program():
    global _NC_CACHE
    if _NC_CACHE is None:
        _NC_CACHE = _build_program()
    return _NC_CACHE


def _make_in_maps(x, W_k, W_q, W_v, W_proj):
    import ml_dtypes

    att_np = ml_dtypes.bfloat16 if ATT_DT == BF16 else np.float32
    mask = np.triu(np.ones((SB, SB), dtype=att_np))  # mask[s,t]=1 iff s<=t
    in_maps = []
    for c in range(N_CORES):
        b, hg = c // 4, c % 4
        rows = slice(hg * HD, (hg + 1) * HD)
        in_maps.append(
            {
                "xT": np.ascontiguousarray(x[b].T).astype(np.float32),
                "wqT": np.ascontiguousarray(W_q[rows].T).astype(np.float32),
                "wkT": np.ascontiguousarray(W_k[rows].T).astype(np.float32),
                "wvT": np.ascontiguousarray(W_v[rows].T).astype(np.float32),
                "wpT": np.ascontiguousarray(W_proj[:, rows].T).astype(np.float32),
                "mask": mask,
                "ones": np.ones((SB, 1), dtype=att_np),
            }
        )
    return in_maps


def _run(x, W_k, W_q, W_v, W_proj, **spmd_kwargs):
    nc = _get_program()
    in_maps = _make_in_maps(x, W_k, W_q, W_v, W_proj)
    res = run_bass_kernel_spmd(nc, in_maps, list(range(N_CORES)), **spmd_kwargs)
    ys = [res.results[c]["y"] for c in range(N_CORES)]
    out = np.stack(
        [
            ys[0] + ys[1] + ys[2] + ys[3],
            ys[4] + ys[5] + ys[6] + ys[7],
        ]
    ).astype(np.float32)
    return out, res


def kernel(x, W_k, W_q, W_v, W_proj):
    out, _ = _run(
        np.asarray(x), np.asarray(W_k), np.asarray(W_q), np.asarray(W_v),
        np.asarray(W_proj),
    )
    return out


# revision 13
# speedup vs baseline: 1.1594x; 1.1594x over previous
"""Causal multi-head attention (B=2, T=2048, C=1024, H=16, D=64) on 8 TRN2 cores.

Sharding: core c -> batch b = c//4, head-group hg = c%4 (4 heads/core).
Each core computes its 4 heads' attention and a partial output projection
(contraction over its 256 feature columns of W_proj); the host sums the 4
partials per batch.

All device-side layouts are transposed on host so the kernel needs no
on-device transposes:
  xT  [C, T]   = x[b].T
  wqT/wkT/wvT [C, 256] = W_{q,k,v}[rows].T
  wpT [256, C] = W_proj[:, rows].T
Attention math per head (D=64):
  qT/kT [64, T] = (wqT chunk).T @ xT            (PE, f32r)
  v     [T, 64] = (xT chunk).T @ wvT            (+ ones col -> row sums)
  sT    [s, t]  = kT.T @ qT                     (K=64)
  pT            = exp(sT/8)  (no max-subtraction needed: |scores/8| < ~7)
  causal: tri-mask on diagonal 128-blocks only; lower kb blocks restricted
  oT_aug[65, t] = v_aug.T @ pT    (row 64 = softmax sums, free)
  oT_norm       = oT * (1/sums)   (reciprocal + DMA partition-broadcast + mul)
  y     [t, p]  = oT.T @ wpT      (partial over this core's 256 features)
"""

import sys

sys.path.insert(0, "/opt/trn_rl_repo")

import numpy as np

import concourse.bass as bass  # noqa: E402
import concourse.mybir as mybir  # noqa: E402
import concourse.tile as tile  # noqa: E402
from concourse import bacc  # noqa: E402
from concourse.bass_utils import run_bass_kernel_spmd  # noqa: E402

F32 = mybir.dt.float32
F32R = mybir.dt.float32r

T = 2048
C = 1024
HL = 4  # heads per core
D = 64
HD = HL * D  # 256 local feature dim
TC = 512  # t-chunk for attention
NTC = T // TC  # 4
SB = 128  # s block
NSB = T // SB  # 16
N_CORES = 8

# matmul input dtype: float32r streams 1 row/cycle at N>=256 (4x faster than
# plain float32) with near-fp32 accuracy (fp32 accumulate).
MM_DT = F32R
BF16 = mybir.dt.bfloat16
# attention (scores/PV) operand dtype
ATT_DT = BF16


def _mm(ap):
    return ap


def _build_program():
    nc = bacc.Bacc("TRN2", target_bir_lowering=False, debug=False)

    xT_d = nc.dram_tensor("xT", [C, T], MM_DT, kind="ExternalInput")
    wqT_d = nc.dram_tensor("wqT", [C, HD], MM_DT, kind="ExternalInput")
    wkT_d = nc.dram_tensor("wkT", [C, HD], MM_DT, kind="ExternalInput")
    wvT_d = nc.dram_tensor("wvT", [C, HD], MM_DT, kind="ExternalInput")
    wpT_d = nc.dram_tensor("wpT", [HD, C], MM_DT, kind="ExternalInput")
    mask_d = nc.dram_tensor("mask", [SB, SB], ATT_DT, kind="ExternalInput")
    ones_d = nc.dram_tensor("ones", [SB, 1], ATT_DT, kind="ExternalInput")
    y_d = nc.dram_tensor("y", [T, C], F32, kind="ExternalOutput")

    NKC = C // SB  # 8 contraction chunks of 128

    with tile.TileContext(nc) as tc:
        with (
            tc.tile_pool(name="persist", bufs=1) as persist,
            tc.tile_pool(name="pt", bufs=10) as pt_pool,
            tc.tile_pool(name="ysb", bufs=3) as ysb_pool,
            tc.tile_pool(name="xtj", bufs=2) as xtj_pool,
            tc.tile_pool(name="norm", bufs=2) as norm_pool,
            tc.tile_pool(name="ps_s", bufs=4, space="PSUM") as ps_s,
            tc.tile_pool(name="ps_o", bufs=4, space="PSUM") as ps_o,
        ):
            ps_mm = ps_s
            ps_y = ps_s
            # ---- persistent SBUF tiles (packed [128, nchunks*width]) ----
            wq_sb = persist.tile([SB, NKC * HD], MM_DT)
            wk_sb = persist.tile([SB, NKC * HD], MM_DT)
            wv_sb = persist.tile([SB, NKC * HD], MM_DT)
            wp_sb = persist.tile([SB, (HD // SB) * C], MM_DT)  # 2 chunks of [128, 1024]
            qT_sb = persist.tile([SB, 2 * T], ATT_DT)  # grp g: heads 2g,2g+1
            kTp_sb = persist.tile([SB, HL * T], ATT_DT)  # head h: cols h*T, rows 64*(h%2), rest zero
            v_sb = persist.tile([SB, NSB * HL * (D + 1)], ATT_DT)  # s-chunk n: n*260
            oT_sb = persist.tile([SB, 2 * T], MM_DT)
            mask_sb = persist.tile([SB, SB], ATT_DT)

            def load_packed(sb, dram_ap, width):
                n = dram_ap.shape[0] // SB
                nc.sync.dma_start(
                    sb[:].rearrange("p (n w) -> p n w", n=n),
                    dram_ap.rearrange("(n p) w -> p n w", p=SB),
                )

            load_packed(wq_sb, wqT_d.ap(), HD)
            load_packed(wk_sb, wkT_d.ap(), HD)
            load_packed(wv_sb, wvT_d.ap(), HD)
            load_packed(wp_sb, wpT_d.ap(), C)
            nc.sync.dma_start(mask_sb[:], mask_d.ap())

            nc.vector.memset(kTp_sb[:], 0.0)

            # ones columns of v_aug (col 64 of each head's 65-col group)
            v_ones = v_sb[:].rearrange("p (k d) -> p k d", k=NSB * HL)[:, :, D : D + 1]
            ones_src = ones_d.ap().unsqueeze(1).to_broadcast((SB, NSB * HL, 1))
            nc.sync.dma_start(v_ones, ones_src)

            # ---- phase 2: QKV projections (xT streamed per t-slice j) ----
            xT_packed = xT_d.ap().rearrange("(n p) t -> p n t", p=SB)
            for j in range(NTC):
                xTj = xtj_pool.tile([SB, NKC * TC], MM_DT, tag="xtj", name=f"xtj_{j}")
                nc.sync.dma_start(
                    xTj[:].rearrange("p (n w) -> p n w", n=NKC),
                    xT_packed[:, :, j * TC : (j + 1) * TC],
                )
                for which, w_sb in (("q", wq_sb), ("k", wk_sb)):
                    for g in range(2):  # partition group (2 heads each)
                        ps = ps_mm.tile([SB, TC], F32, tag="s")
                        for n in range(NKC):
                            nc.tensor.matmul(
                                ps[:],
                                _mm(w_sb[:, n * HD + g * SB : n * HD + (g + 1) * SB]),
                                _mm(xTj[:, n * TC : (n + 1) * TC]),
                                start=(n == 0),
                                stop=(n == NKC - 1),
                            )
                        if which == "q":
                            nc.scalar.copy(
                                qT_sb[:, g * T + j * TC : g * T + (j + 1) * TC], ps[:]
                            )
                        else:
                            # head 2g -> rows 0:64, head 2g+1 -> rows 64:128
                            for hh in range(2):
                                h = 2 * g + hh
                                nc.vector.tensor_copy(
                                    kTp_sb[
                                        hh * D : (hh + 1) * D,
                                        h * T + j * TC : h * T + (j + 1) * TC,
                                    ],
                                    ps[hh * D : (hh + 1) * D, :],
                                )

                for n in range(HL * j, HL * j + HL):  # v: out [128 s, 256 d]
                    off = (n - HL * j) * SB
                    ps = ps_mm.tile([SB, TC], F32, tag="s")
                    for m in range(NKC):
                        nc.tensor.matmul(
                            ps[:, 0:HD],
                            _mm(xTj[:, m * TC + off : m * TC + off + SB]),
                            _mm(wv_sb[:, m * HD : (m + 1) * HD]),
                            start=(m == 0),
                            stop=(m == NKC - 1),
                        )
                    dst = v_sb[:, n * HL * (D + 1) : (n + 1) * HL * (D + 1)].rearrange(
                        "p (h d) -> p h d", h=HL
                    )[:, :, 0:D]
                    src = ps[:, 0:HD].rearrange("p (h d) -> p h d", h=HL)
                    nc.scalar.copy(dst, src)

            # ---- phase 3: attention, 4 heads interleaved per (j, kb) ----
            # scores for head pairs run concurrently in disjoint PE row groups
            for j in range(NTC):
                po = [
                    ps_o.tile([D + 1, TC], F32, tag="o", name=f"po_{j}_{h}")
                    for h in range(HL)
                ]
                last_kb = HL * j + 3
                for kb in range(last_kb + 1):
                    tstart = max(0, (kb - HL * j) * SB)
                    pss = [
                        ps_s.tile([SB, TC], F32, tag="s", name=f"pss_{j}_{kb}_{h}")
                        for h in range(HL)
                    ]
                    pT = [
                        pt_pool.tile([SB, TC], ATT_DT, tag="pt", name=f"pt_{j}_{kb}_{h}")
                        for h in range(HL)
                    ]
                    for h in range(HL):
                        hg = h // 2
                        nc.tensor.matmul(
                            pss[h][:, tstart:],
                            _mm(kTp_sb[:, h * T + kb * SB : h * T + (kb + 1) * SB]),
                            _mm(
                                qT_sb[
                                    :,
                                    hg * T + j * TC + tstart : hg * T + (j + 1) * TC,
                                ]
                            ),
                            start=True,
                            stop=True,
                        )
                    for h in range(HL):
                        nc.scalar.activation(
                            pT[h][:, tstart:],
                            pss[h][:, tstart:],
                            mybir.ActivationFunctionType.Exp,
                            scale=float(D) ** -0.5,
                        )
                        if kb >= HL * j:  # diagonal block: causal tri-mask
                            nc.vector.tensor_mul(
                                pT[h][:, tstart : tstart + SB],
                                pT[h][:, tstart : tstart + SB],
                                mask_sb[:],
                            )
                    for h in range(HL):
                        nc.tensor.matmul(
                            po[h][:, tstart:],
                            _mm(
                                v_sb[
                                    :,
                                    kb * HL * (D + 1)
                                    + h * (D + 1) : kb * HL * (D + 1)
                                    + (h + 1) * (D + 1),
                                ]
                            ),
                            _mm(pT[h][:, tstart:]),
                            start=(kb == 0),
                            stop=(kb == last_kb),
                        )
                # normalize: oT_sb[...] = po[0:64] * (1 / po[64])
                for h in range(HL):
                    hp = D * (h % 2)
                    hg = h // 2
                    row = norm_pool.tile([1, TC], F32, tag="row", name=f"row_{j}_{h}")
                    nc.vector.reciprocal(row[:], po[h][D : D + 1, :])
                    bc = norm_pool.tile([D, TC], F32, tag="bc", name=f"bc_{j}_{h}")
                    nc.gpsimd.partition_broadcast(bc[:], row[:])
                    nc.vector.tensor_mul(
                        oT_sb[hp : hp + D, hg * T + j * TC : hg * T + (j + 1) * TC],
                        po[h][0:D, :],
                        bc[:],
                    )

            # ---- phase 4: output projection ----
            for i in range(NSB):
                for half in range(2):
                    ps = ps_y.tile([SB, TC], F32, tag="s")
                    for g in range(2):
                        nc.tensor.matmul(
                            ps[:],
                            _mm(oT_sb[:, g * T + i * SB : g * T + (i + 1) * SB]),
                            _mm(wp_sb[:, g * C + half * TC : g * C + (half + 1) * TC]),
                            start=(g == 0),
                            stop=(g == 1),
                        )
                    y_sb = ysb_pool.tile([SB, TC], F32, tag="ysb")
                    nc.vector.tensor_copy(y_sb[:], ps[:])
                    nc.sync.dma_start(
                        y_d.ap()[i * SB : (i + 1) * SB, half * TC : (half + 1) * TC],
                        y_sb[:],
                    )

    nc.compile()
    return nc


_NC_CACHE = None


def _get_program():
    global _NC_CACHE
    if _NC_CACHE is None:
        _NC_CACHE = _build_program()
    return _NC_CACHE


def _make_in_maps(x, W_k, W_q, W_v, W_proj):
    import ml_dtypes

    att_np = ml_dtypes.bfloat16 if ATT_DT == BF16 else np.float32
    mask = np.triu(np.ones((SB, SB), dtype=att_np))  # mask[s,t]=1 iff s<=t
    in_maps = []
    for c in range(N_CORES):
        b, hg = c // 4, c % 4
        rows = slice(hg * HD, (hg + 1) * HD)
        in_maps.append(
            {
                "xT": np.ascontiguousarray(x[b].T).astype(np.float32),
                "wqT": np.ascontiguousarray(W_q[rows].T).astype(np.float32),
                "wkT": np.ascontiguousarray(W_k[rows].T).astype(np.float32),
                "wvT": np.ascontiguousarray(W_v[rows].T).astype(np.float32),
                "wpT": np.ascontiguousarray(W_proj[:, rows].T).astype(np.float32),
                "mask": mask,
                "ones": np.ones((SB, 1), dtype=att_np),
            }
        )
    return in_maps


def _run(x, W_k, W_q, W_v, W_proj, **spmd_kwargs):
    nc = _get_program()
    in_maps = _make_in_maps(x, W_k, W_q, W_v, W_proj)
    res = run_bass_kernel_spmd(nc, in_maps, list(range(N_CORES)), **spmd_kwargs)
    ys = [res.results[c]["y"] for c in range(N_CORES)]
    out = np.stack(
        [
            ys[0] + ys[1] + ys[2] + ys[3],
            ys[4] + ys[5] + ys[6] + ys[7],
        ]
    ).astype(np.float32)
    return out, res


def kernel(x, W_k, W_q, W_v, W_proj):
    out, _ = _run(
        np.asarray(x), np.asarray(W_k), np.asarray(W_q), np.asarray(W_v),
        np.asarray(W_proj),
    )
    return out


# revision 15
# speedup vs baseline: 1.2201x; 1.0524x over previous
"""Causal multi-head attention (B=2, T=2048, C=1024, H=16, D=64) on 8 TRN2 cores.

Sharding: core c -> batch b = c//4, head-group hg = c%4 (4 heads/core).
Each core computes its 4 heads' attention and a partial output projection
(contraction over its 256 feature columns of W_proj); the host sums the 4
partials per batch.

All device-side layouts are transposed on host so the kernel needs no
on-device transposes:
  xT  [C, T]   = x[b].T
  wqT/wkT/wvT [C, 256] = W_{q,k,v}[rows].T
  wpT [256, C] = W_proj[:, rows].T
Attention math per head (D=64):
  qT/kT [64, T] = (wqT chunk).T @ xT            (PE, f32r)
  v     [T, 64] = (xT chunk).T @ wvT            (+ ones col -> row sums)
  sT    [s, t]  = kT.T @ qT                     (K=64)
  pT            = exp(sT/8)  (no max-subtraction needed: |scores/8| < ~7)
  causal: tri-mask on diagonal 128-blocks only; lower kb blocks restricted
  oT_aug[65, t] = v_aug.T @ pT    (row 64 = softmax sums, free)
  oT_norm       = oT * (1/sums)   (reciprocal + DMA partition-broadcast + mul)
  y     [t, p]  = oT.T @ wpT      (partial over this core's 256 features)
"""

import sys

sys.path.insert(0, "/opt/trn_rl_repo")

import numpy as np

import concourse.bass as bass  # noqa: E402
import concourse.mybir as mybir  # noqa: E402
import concourse.tile as tile  # noqa: E402
from concourse import bacc  # noqa: E402
from concourse.bass_utils import run_bass_kernel_spmd  # noqa: E402

F32 = mybir.dt.float32
F32R = mybir.dt.float32r

T = 2048
C = 1024
HL = 4  # heads per core
D = 64
HD = HL * D  # 256 local feature dim
TC = 512  # t-chunk for attention
NTC = T // TC  # 4
SB = 128  # s block
NSB = T // SB  # 16
N_CORES = 8

# matmul input dtype: float32r streams 1 row/cycle at N>=256 (4x faster than
# plain float32) with near-fp32 accuracy (fp32 accumulate).
MM_DT = F32R
BF16 = mybir.dt.bfloat16
# attention (scores/PV) operand dtype
ATT_DT = BF16


def _mm(ap):
    return ap


def _build_program():
    nc = bacc.Bacc("TRN2", target_bir_lowering=False, debug=False)

    xT_d = nc.dram_tensor("xT", [C, T], MM_DT, kind="ExternalInput")
    wqT_d = nc.dram_tensor("wqT", [C, HD], MM_DT, kind="ExternalInput")
    wkT_d = nc.dram_tensor("wkT", [C, HD], MM_DT, kind="ExternalInput")
    wvT_d = nc.dram_tensor("wvT", [C, HD], MM_DT, kind="ExternalInput")
    wpT_d = nc.dram_tensor("wpT", [HD, C], MM_DT, kind="ExternalInput")
    mask_d = nc.dram_tensor("mask", [SB, SB], ATT_DT, kind="ExternalInput")
    ones_d = nc.dram_tensor("ones", [SB, 1], ATT_DT, kind="ExternalInput")
    y_d = nc.dram_tensor("y", [T, C], F32, kind="ExternalOutput")

    NKC = C // SB  # 8 contraction chunks of 128

    with tile.TileContext(nc) as tc:
        with (
            tc.tile_pool(name="persist", bufs=1) as persist,
            tc.tile_pool(name="pt", bufs=10) as pt_pool,
            tc.tile_pool(name="ysb", bufs=3) as ysb_pool,
            tc.tile_pool(name="xtj", bufs=2) as xtj_pool,
            tc.tile_pool(name="norm", bufs=2) as norm_pool,
            tc.tile_pool(name="ps_s", bufs=4, space="PSUM") as ps_s,
            tc.tile_pool(name="ps_o", bufs=4, space="PSUM") as ps_o,
        ):
            ps_mm = ps_s
            ps_y = ps_s
            # ---- persistent SBUF tiles (packed [128, nchunks*width]) ----
            wq_sb = persist.tile([SB, NKC * HD], MM_DT)
            wk_sb = persist.tile([SB, NKC * HD], MM_DT)
            wv_sb = persist.tile([SB, NKC * HD], MM_DT)
            wp_sb = persist.tile([SB, (HD // SB) * C], MM_DT)  # 2 chunks of [128, 1024]
            qT_sb = persist.tile([SB, 2 * T], ATT_DT)  # grp g: heads 2g,2g+1
            kTp_sb = persist.tile([SB, HL * T], ATT_DT)  # head h: cols h*T, rows 64*(h%2), rest zero
            v_sb = persist.tile([SB, NSB * HL * SB], ATT_DT)  # (chunk n, head h): cols (n*HL+h)*128
            oT_sb = persist.tile([SB, 2 * T], MM_DT)
            mask_sb = persist.tile([SB, SB], ATT_DT)

            def load_packed(sb, dram_ap, width):
                n = dram_ap.shape[0] // SB
                nc.sync.dma_start(
                    sb[:].rearrange("p (n w) -> p n w", n=n),
                    dram_ap.rearrange("(n p) w -> p n w", p=SB),
                )

            load_packed(wq_sb, wqT_d.ap(), HD)
            load_packed(wk_sb, wkT_d.ap(), HD)
            load_packed(wv_sb, wvT_d.ap(), HD)
            load_packed(wp_sb, wpT_d.ap(), C)
            nc.sync.dma_start(mask_sb[:], mask_d.ap())

            nc.vector.memset(kTp_sb[:], 0.0)
            nc.vector.memset(v_sb[:], 0.0)

            # ones columns of v_aug (col 64 of each head's 65-col group)
            v_ones = v_sb[:].rearrange("p (k d) -> p k d", k=NSB * HL)[:, :, D : D + 1]  # d=128 now
            ones_src = ones_d.ap().unsqueeze(1).to_broadcast((SB, NSB * HL, 1))
            nc.sync.dma_start(v_ones, ones_src)

            # ---- phase 2: QKV projections (xT streamed per t-slice j) ----
            xT_packed = xT_d.ap().rearrange("(n p) t -> p n t", p=SB)
            for j in range(NTC):
                xTj = xtj_pool.tile([SB, NKC * TC], MM_DT, tag="xtj", name=f"xtj_{j}")
                nc.sync.dma_start(
                    xTj[:].rearrange("p (n w) -> p n w", n=NKC),
                    xT_packed[:, :, j * TC : (j + 1) * TC],
                )
                for which, w_sb in (("q", wq_sb), ("k", wk_sb)):
                    for g in range(2):  # partition group (2 heads each)
                        ps = ps_mm.tile([SB, TC], F32, tag="s")
                        for n in range(NKC):
                            nc.tensor.matmul(
                                ps[:],
                                _mm(w_sb[:, n * HD + g * SB : n * HD + (g + 1) * SB]),
                                _mm(xTj[:, n * TC : (n + 1) * TC]),
                                start=(n == 0),
                                stop=(n == NKC - 1),
                            )
                        if which == "q":
                            nc.scalar.copy(
                                qT_sb[:, g * T + j * TC : g * T + (j + 1) * TC], ps[:]
                            )
                        else:
                            # head 2g -> rows 0:64, head 2g+1 -> rows 64:128
                            for hh in range(2):
                                h = 2 * g + hh
                                nc.vector.tensor_copy(
                                    kTp_sb[
                                        hh * D : (hh + 1) * D,
                                        h * T + j * TC : h * T + (j + 1) * TC,
                                    ],
                                    ps[hh * D : (hh + 1) * D, :],
                                )

                for n in range(HL * j, HL * j + HL):  # v: out [128 s, 256 d]
                    off = (n - HL * j) * SB
                    ps = ps_mm.tile([SB, TC], F32, tag="s")
                    for m in range(NKC):
                        nc.tensor.matmul(
                            ps[:, 0:HD],
                            _mm(xTj[:, m * TC + off : m * TC + off + SB]),
                            _mm(wv_sb[:, m * HD : (m + 1) * HD]),
                            start=(m == 0),
                            stop=(m == NKC - 1),
                        )
                    dst = v_sb[:, n * HL * SB : (n + 1) * HL * SB].rearrange(
                        "p (h d) -> p h d", h=HL
                    )[:, :, 0:D]
                    src = ps[:, 0:HD].rearrange("p (h d) -> p h d", h=HL)
                    nc.scalar.copy(dst, src)

            # ---- phase 3: attention, 4 heads interleaved per (j, kb) ----
            # scores for head pairs run concurrently in disjoint PE row groups
            for j in range(NTC):
                po = [
                    ps_o.tile([SB, TC], F32, tag="o", name=f"po_{j}_{h}")
                    for h in range(HL)
                ]
                last_kb = HL * j + 3
                for kb in range(last_kb + 1):
                    tstart = max(0, (kb - HL * j) * SB)
                    pss = [
                        ps_s.tile([SB, TC], F32, tag="s", name=f"pss_{j}_{kb}_{h}")
                        for h in range(HL)
                    ]
                    pT = [
                        pt_pool.tile([SB, TC], ATT_DT, tag="pt", name=f"pt_{j}_{kb}_{h}")
                        for h in range(HL)
                    ]
                    for h in range(HL):
                        hg = h // 2
                        nc.tensor.matmul(
                            pss[h][:, tstart:],
                            _mm(kTp_sb[:, h * T + kb * SB : h * T + (kb + 1) * SB]),
                            _mm(
                                qT_sb[
                                    :,
                                    hg * T + j * TC + tstart : hg * T + (j + 1) * TC,
                                ]
                            ),
                            start=True,
                            stop=True,
                        )
                    for h in range(HL):
                        nc.scalar.activation(
                            pT[h][:, tstart:],
                            pss[h][:, tstart:],
                            mybir.ActivationFunctionType.Exp,
                            scale=float(D) ** -0.5,
                        )
                        if kb >= HL * j:  # diagonal block: causal tri-mask
                            nc.vector.tensor_mul(
                                pT[h][:, tstart : tstart + SB],
                                pT[h][:, tstart : tstart + SB],
                                mask_sb[:],
                            )
                    for h in range(HL):
                        nc.tensor.matmul(
                            po[h][:, tstart:],
                            _mm(v_sb[:, (kb * HL + h) * SB : (kb * HL + h + 1) * SB]),
                            _mm(pT[h][:, tstart:]),
                            start=(kb == 0),
                            stop=(kb == last_kb),
                        )
                # normalize: oT_sb[...] = po[0:64] * (1 / po[64])
                for h in range(HL):
                    hp = D * (h % 2)
                    hg = h // 2
                    tmp = norm_pool.tile(
                        [D, TC], F32, tag="tmp", name=f"tmp_{j}_{h}"
                    )
                    nc.scalar.copy(tmp[:], po[h][0:D, :])
                    row = norm_pool.tile([1, TC], F32, tag="row", name=f"row_{j}_{h}")
                    nc.scalar.copy(row[:], po[h][D : D + 1, :])  # releases po
                    bc = norm_pool.tile([D, TC], F32, tag="bc", name=f"bc_{j}_{h}")
                    nc.gpsimd.partition_broadcast(bc[:], row[:])
                    nc.vector.reciprocal(bc[:], bc[:])
                    nc.vector.tensor_mul(
                        oT_sb[hp : hp + D, hg * T + j * TC : hg * T + (j + 1) * TC],
                        tmp[:],
                        bc[:],
                    )

            # ---- phase 4: output projection ----
            for i in range(NSB):
                for half in range(2):
                    ps = ps_y.tile([SB, TC], F32, tag="s")
                    for g in range(2):
                        nc.tensor.matmul(
                            ps[:],
                            _mm(oT_sb[:, g * T + i * SB : g * T + (i + 1) * SB]),
                            _mm(wp_sb[:, g * C + half * TC : g * C + (half + 1) * TC]),
                            start=(g == 0),
                            stop=(g == 1),
                        )
                    y_sb = ysb_pool.tile([SB, TC], F32, tag="ysb")
                    nc.vector.tensor_copy(y_sb[:], ps[:])
                    nc.sync.dma_start(
                        y_d.ap()[i * SB : (i + 1) * SB, half * TC : (half + 1) * TC],
                        y_sb[:],
                    )

    nc.compile()
    return nc


_NC_CACHE = None


def _get_program():
    global _NC_CACHE
    if _NC_CACHE is None:
        _NC_CACHE = _build_program()
    return _NC_CACHE


def _make_in_maps(x, W_k, W_q, W_v, W_proj):
    import ml_dtypes

    att_np = ml_dtypes.bfloat16 if ATT_DT == BF16 else np.float32
    mask = np.triu(np.ones((SB, SB), dtype=att_np))  # mask[s,t]=1 iff s<=t
    in_maps = []
    for c in range(N_CORES):
        b, hg = c // 4, c % 4
        rows = slice(hg * HD, (hg + 1) * HD)
        in_maps.append(
            {
                "xT": np.ascontiguousarray(x[b].T).astype(np.float32),
                "wqT": np.ascontiguousarray(W_q[rows].T).astype(np.float32),
                "wkT": np.ascontiguousarray(W_k[rows].T).astype(np.float32),
                "wvT": np.ascontiguousarray(W_v[rows].T).astype(np.float32),
                "wpT": np.ascontiguousarray(W_proj[:, rows].T).astype(np.float32),
                "mask": mask,
                "ones": np.ones((SB, 1), dtype=att_np),
            }
        )
    return in_maps


def _run(x, W_k, W_q, W_v, W_proj, **spmd_kwargs):
    nc = _get_program()
    in_maps = _make_in_maps(x, W_k, W_q, W_v, W_proj)
    res = run_bass_kernel_spmd(nc, in_maps, list(range(N_CORES)), **spmd_kwargs)
    ys = [res.results[c]["y"] for c in range(N_CORES)]
    out = np.stack(
        [
            ys[0] + ys[1] + ys[2] + ys[3],
            ys[4] + ys[5] + ys[6] + ys[7],
        ]
    ).astype(np.float32)
    return out, res


def kernel(x, W_k, W_q, W_v, W_proj):
    out, _ = _run(
        np.asarray(x), np.asarray(W_k), np.asarray(W_q), np.asarray(W_v),
        np.asarray(W_proj),
    )
    return out


# revision 16
# speedup vs baseline: 1.5369x; 1.2596x over previous
"""Causal multi-head attention (B=2, T=2048, C=1024, H=16, D=64) on 8 TRN2 cores.

Sharding: core c -> batch b = c//4, head-group hg = c%4 (4 heads/core).
Each core computes its 4 heads' attention and a partial output projection
(contraction over its 256 feature columns of W_proj); the host sums the 4
partials per batch.

All device-side layouts are transposed on host so the kernel needs no
on-device transposes:
  xT  [C, T]   = x[b].T
  wqT/wkT/wvT [C, 256] = W_{q,k,v}[rows].T
  wpT [256, C] = W_proj[:, rows].T
Attention math per head (D=64):
  qT/kT [64, T] = (wqT chunk).T @ xT            (PE, f32r)
  v     [T, 64] = (xT chunk).T @ wvT            (+ ones col -> row sums)
  sT    [s, t]  = kT.T @ qT                     (K=64)
  pT            = exp(sT/8)  (no max-subtraction needed: |scores/8| < ~7)
  causal: tri-mask on diagonal 128-blocks only; lower kb blocks restricted
  oT_aug[65, t] = v_aug.T @ pT    (row 64 = softmax sums, free)
  oT_norm       = oT * (1/sums)   (reciprocal + DMA partition-broadcast + mul)
  y     [t, p]  = oT.T @ wpT      (partial over this core's 256 features)
"""

import sys

sys.path.insert(0, "/opt/trn_rl_repo")

import numpy as np

import concourse.bass as bass  # noqa: E402
import concourse.mybir as mybir  # noqa: E402
import concourse.tile as tile  # noqa: E402
from concourse import bacc  # noqa: E402
from concourse.bass_utils import run_bass_kernel_spmd  # noqa: E402

F32 = mybir.dt.float32
F32R = mybir.dt.float32r

T = 2048
C = 1024
HL = 4  # heads per core
D = 64
HD = HL * D  # 256 local feature dim
TC = 512  # t-chunk for attention
NTC = T // TC  # 4
SB = 128  # s block
NSB = T // SB  # 16
N_CORES = 8

# matmul input dtype: float32r streams 1 row/cycle at N>=256 (4x faster than
# plain float32) with near-fp32 accuracy (fp32 accumulate).
MM_DT = F32R
BF16 = mybir.dt.bfloat16
# attention (scores/PV) operand dtype
ATT_DT = BF16


def _mm(ap):
    return ap


def _build_program():
    nc = bacc.Bacc("TRN2", target_bir_lowering=False, debug=False)

    xT_d = nc.dram_tensor("xT", [C, T], MM_DT, kind="ExternalInput")
    wqT_d = nc.dram_tensor("wqT", [C, HD], MM_DT, kind="ExternalInput")
    wkT_d = nc.dram_tensor("wkT", [C, HD], MM_DT, kind="ExternalInput")
    wvT_d = nc.dram_tensor("wvT", [C, HD], MM_DT, kind="ExternalInput")
    wpT_d = nc.dram_tensor("wpT", [HD, C], MM_DT, kind="ExternalInput")
    mask_d = nc.dram_tensor("mask", [SB, SB], ATT_DT, kind="ExternalInput")
    ones_d = nc.dram_tensor("ones", [SB, NSB * HL], ATT_DT, kind="ExternalInput")
    y_d = nc.dram_tensor("y", [T, C], F32, kind="ExternalOutput")

    NKC = C // SB  # 8 contraction chunks of 128

    with tile.TileContext(nc) as tc:
        with (
            tc.tile_pool(name="persist", bufs=1) as persist,
            tc.tile_pool(name="pt", bufs=10) as pt_pool,
            tc.tile_pool(name="ysb", bufs=3) as ysb_pool,
            tc.tile_pool(name="xtj", bufs=2) as xtj_pool,
            tc.tile_pool(name="norm", bufs=2) as norm_pool,
            tc.tile_pool(name="ps_s", bufs=4, space="PSUM") as ps_s,
            tc.tile_pool(name="ps_o", bufs=4, space="PSUM") as ps_o,
        ):
            ps_mm = ps_s
            ps_y = ps_s
            # ---- persistent SBUF tiles (packed [128, nchunks*width]) ----
            wq_sb = persist.tile([SB, NKC * HD], MM_DT)
            wk_sb = persist.tile([SB, NKC * HD], MM_DT)
            wv_sb = persist.tile([SB, NKC * HD], MM_DT)
            wp_sb = persist.tile([SB, (HD // SB) * C], MM_DT)  # 2 chunks of [128, 1024]
            qT_sb = persist.tile([SB, 2 * T], ATT_DT)  # grp g: heads 2g,2g+1
            kTp_sb = persist.tile([SB, HL * T], ATT_DT)  # head h: cols h*T, rows 64*(h%2), rest zero
            v_sb = persist.tile([SB, NSB * HL * SB], ATT_DT)  # (chunk n, head h): cols (n*HL+h)*128
            oT_sb = persist.tile([SB, 2 * T], MM_DT)
            mask_sb = persist.tile([SB, SB], ATT_DT)

            def load_packed(sb, dram_ap, width):
                n = dram_ap.shape[0] // SB
                nc.sync.dma_start(
                    sb[:].rearrange("p (n w) -> p n w", n=n),
                    dram_ap.rearrange("(n p) w -> p n w", p=SB),
                )

            load_packed(wq_sb, wqT_d.ap(), HD)
            load_packed(wk_sb, wkT_d.ap(), HD)
            load_packed(wv_sb, wvT_d.ap(), HD)
            load_packed(wp_sb, wpT_d.ap(), C)
            nc.sync.dma_start(mask_sb[:], mask_d.ap())

            nc.vector.memset(kTp_sb[:], 0.0)
            nc.vector.memset(v_sb[:], 0.0)

            # ones columns of v_aug (col 64 of each head's 65-col group)
            v_ones = v_sb[:].rearrange("p (k d) -> p k d", k=NSB * HL)[:, :, D : D + 1]  # d=128 now
            nc.sync.dma_start(v_ones, ones_d.ap().unsqueeze(2))

            # ---- phase 2: QKV projections (xT streamed per t-slice j) ----
            xT_packed = xT_d.ap().rearrange("(n p) t -> p n t", p=SB)
            for j in range(NTC):
                xTj = xtj_pool.tile([SB, NKC * TC], MM_DT, tag="xtj", name=f"xtj_{j}")
                nc.sync.dma_start(
                    xTj[:].rearrange("p (n w) -> p n w", n=NKC),
                    xT_packed[:, :, j * TC : (j + 1) * TC],
                )
                for which, w_sb in (("q", wq_sb), ("k", wk_sb)):
                    for g in range(2):  # partition group (2 heads each)
                        ps = ps_mm.tile([SB, TC], F32, tag="s")
                        for n in range(NKC):
                            nc.tensor.matmul(
                                ps[:],
                                _mm(w_sb[:, n * HD + g * SB : n * HD + (g + 1) * SB]),
                                _mm(xTj[:, n * TC : (n + 1) * TC]),
                                start=(n == 0),
                                stop=(n == NKC - 1),
                            )
                        if which == "q":
                            nc.scalar.copy(
                                qT_sb[:, g * T + j * TC : g * T + (j + 1) * TC], ps[:]
                            )
                        else:
                            # head 2g -> rows 0:64, head 2g+1 -> rows 64:128
                            for hh in range(2):
                                h = 2 * g + hh
                                nc.vector.tensor_copy(
                                    kTp_sb[
                                        hh * D : (hh + 1) * D,
                                        h * T + j * TC : h * T + (j + 1) * TC,
                                    ],
                                    ps[hh * D : (hh + 1) * D, :],
                                )

                for n in range(HL * j, HL * j + HL):  # v: out [128 s, 256 d]
                    off = (n - HL * j) * SB
                    ps = ps_mm.tile([SB, TC], F32, tag="s")
                    for m in range(NKC):
                        nc.tensor.matmul(
                            ps[:, 0:HD],
                            _mm(xTj[:, m * TC + off : m * TC + off + SB]),
                            _mm(wv_sb[:, m * HD : (m + 1) * HD]),
                            start=(m == 0),
                            stop=(m == NKC - 1),
                        )
                    dst = v_sb[:, n * HL * SB : (n + 1) * HL * SB].rearrange(
                        "p (h d) -> p h d", h=HL
                    )[:, :, 0:D]
                    src = ps[:, 0:HD].rearrange("p (h d) -> p h d", h=HL)
                    nc.scalar.copy(dst, src)

            # ---- phase 3: attention, 4 heads interleaved per (j, kb) ----
            # scores for head pairs run concurrently in disjoint PE row groups
            for j in range(NTC):
                po = [
                    ps_o.tile([SB, TC], F32, tag="o", name=f"po_{j}_{h}")
                    for h in range(HL)
                ]
                last_kb = HL * j + 3
                for kb in range(last_kb + 1):
                    tstart = max(0, (kb - HL * j) * SB)
                    pss = [
                        ps_s.tile([SB, TC], F32, tag="s", name=f"pss_{j}_{kb}_{h}")
                        for h in range(HL)
                    ]
                    pT = [
                        pt_pool.tile([SB, TC], ATT_DT, tag="pt", name=f"pt_{j}_{kb}_{h}")
                        for h in range(HL)
                    ]
                    for h in range(HL):
                        hg = h // 2
                        nc.tensor.matmul(
                            pss[h][:, tstart:],
                            _mm(kTp_sb[:, h * T + kb * SB : h * T + (kb + 1) * SB]),
                            _mm(
                                qT_sb[
                                    :,
                                    hg * T + j * TC + tstart : hg * T + (j + 1) * TC,
                                ]
                            ),
                            start=True,
                            stop=True,
                        )
                    for h in range(HL):
                        nc.scalar.activation(
                            pT[h][:, tstart:],
                            pss[h][:, tstart:],
                            mybir.ActivationFunctionType.Exp,
                            scale=float(D) ** -0.5,
                        )
                        if kb >= HL * j:  # diagonal block: causal tri-mask
                            nc.vector.tensor_mul(
                                pT[h][:, tstart : tstart + SB],
                                pT[h][:, tstart : tstart + SB],
                                mask_sb[:],
                            )
                    for h in range(HL):
                        nc.tensor.matmul(
                            po[h][:, tstart:],
                            _mm(v_sb[:, (kb * HL + h) * SB : (kb * HL + h + 1) * SB]),
                            _mm(pT[h][:, tstart:]),
                            start=(kb == 0),
                            stop=(kb == last_kb),
                        )
                # normalize: oT_sb[...] = po[0:64] * (1 / po[64])
                for h in range(HL):
                    hp = D * (h % 2)
                    hg = h // 2
                    tmp = norm_pool.tile(
                        [D, TC], F32, tag="tmp", name=f"tmp_{j}_{h}"
                    )
                    nc.scalar.copy(tmp[:], po[h][0:D, :])
                    row = norm_pool.tile([1, TC], F32, tag="row", name=f"row_{j}_{h}")
                    nc.scalar.copy(row[:], po[h][D : D + 1, :])  # releases po
                    bc = norm_pool.tile([D, TC], F32, tag="bc", name=f"bc_{j}_{h}")
                    nc.gpsimd.partition_broadcast(bc[:], row[:])
                    nc.vector.reciprocal(bc[:], bc[:])
                    nc.vector.tensor_mul(
                        oT_sb[hp : hp + D, hg * T + j * TC : hg * T + (j + 1) * TC],
                        tmp[:],
                        bc[:],
                    )

            # ---- phase 4: output projection ----
            for i in range(NSB):
                for half in range(2):
                    ps = ps_y.tile([SB, TC], F32, tag="s")
                    for g in range(2):
                        nc.tensor.matmul(
                            ps[:],
                            _mm(oT_sb[:, g * T + i * SB : g * T + (i + 1) * SB]),
                            _mm(wp_sb[:, g * C + half * TC : g * C + (half + 1) * TC]),
                            start=(g == 0),
                            stop=(g == 1),
                        )
                    y_sb = ysb_pool.tile([SB, TC], F32, tag="ysb")
                    nc.vector.tensor_copy(y_sb[:], ps[:])
                    nc.sync.dma_start(
                        y_d.ap()[i * SB : (i + 1) * SB, half * TC : (half + 1) * TC],
                        y_sb[:],
                    )

    nc.compile()
    return nc


_NC_CACHE = None


def _get_program():
    global _NC_CACHE
    if _NC_CACHE is None:
        _NC_CACHE = _build_program()
    return _NC_CACHE


def _make_in_maps(x, W_k, W_q, W_v, W_proj):
    import ml_dtypes

    att_np = ml_dtypes.bfloat16 if ATT_DT == BF16 else np.float32
    mask = np.triu(np.ones((SB, SB), dtype=att_np))  # mask[s,t]=1 iff s<=t
    in_maps = []
    for c in range(N_CORES):
        b, hg = c // 4, c % 4
        rows = slice(hg * HD, (hg + 1) * HD)
        in_maps.append(
            {
                "xT": np.ascontiguousarray(x[b].T).astype(np.float32),
                "wqT": np.ascontiguousarray(W_q[rows].T).astype(np.float32),
                "wkT": np.ascontiguousarray(W_k[rows].T).astype(np.float32),
                "wvT": np.ascontiguousarray(W_v[rows].T).astype(np.float32),
                "wpT": np.ascontiguousarray(W_proj[:, rows].T).astype(np.float32),
                "mask": mask,
                "ones": np.ones((SB, NSB * HL), dtype=att_np),
            }
        )
    return in_maps


def _run(x, W_k, W_q, W_v, W_proj, **spmd_kwargs):
    nc = _get_program()
    in_maps = _make_in_maps(x, W_k, W_q, W_v, W_proj)
    res = run_bass_kernel_spmd(nc, in_maps, list(range(N_CORES)), **spmd_kwargs)
    ys = [res.results[c]["y"] for c in range(N_CORES)]
    out = np.stack(
        [
            ys[0] + ys[1] + ys[2] + ys[3],
            ys[4] + ys[5] + ys[6] + ys[7],
        ]
    ).astype(np.float32)
    return out, res


def kernel(x, W_k, W_q, W_v, W_proj):
    out, _ = _run(
        np.asarray(x), np.asarray(W_k), np.asarray(W_q), np.asarray(W_v),
        np.asarray(W_proj),
    )
    return out


# revision 17
# speedup vs baseline: 1.5723x; 1.0230x over previous
"""Causal multi-head attention (B=2, T=2048, C=1024, H=16, D=64) on 8 TRN2 cores.

Sharding: core c -> batch b = c//4, head-group hg = c%4 (4 heads/core).
Each core computes its 4 heads' attention and a partial output projection
(contraction over its 256 feature columns of W_proj); the host sums the 4
partials per batch.

All device-side layouts are transposed on host so the kernel needs no
on-device transposes:
  xT  [C, T]   = x[b].T
  wqT/wkT/wvT [C, 256] = W_{q,k,v}[rows].T
  wpT [256, C] = W_proj[:, rows].T
Attention math per head (D=64):
  qT/kT [64, T] = (wqT chunk).T @ xT            (PE, f32r)
  v     [T, 64] = (xT chunk).T @ wvT            (+ ones col -> row sums)
  sT    [s, t]  = kT.T @ qT                     (K=64)
  pT            = exp(sT/8)  (no max-subtraction needed: |scores/8| < ~7)
  causal: tri-mask on diagonal 128-blocks only; lower kb blocks restricted
  oT_aug[65, t] = v_aug.T @ pT    (row 64 = softmax sums, free)
  oT_norm       = oT * (1/sums)   (reciprocal + DMA partition-broadcast + mul)
  y     [t, p]  = oT.T @ wpT      (partial over this core's 256 features)
"""

import sys

sys.path.insert(0, "/opt/trn_rl_repo")

import numpy as np

import concourse.bass as bass  # noqa: E402
import concourse.mybir as mybir  # noqa: E402
import concourse.tile as tile  # noqa: E402
from concourse import bacc  # noqa: E402
from concourse.bass_utils import run_bass_kernel_spmd  # noqa: E402

F32 = mybir.dt.float32
F32R = mybir.dt.float32r

T = 2048
C = 1024
HL = 4  # heads per core
D = 64
HD = HL * D  # 256 local feature dim
TC = 512  # t-chunk for attention
NTC = T // TC  # 4
SB = 128  # s block
NSB = T // SB  # 16
N_CORES = 8

# matmul input dtype: float32r streams 1 row/cycle at N>=256 (4x faster than
# plain float32) with near-fp32 accuracy (fp32 accumulate).
MM_DT = F32R
BF16 = mybir.dt.bfloat16
# attention (scores/PV) operand dtype
ATT_DT = BF16


def _mm(ap):
    return ap


def _build_program():
    nc = bacc.Bacc("TRN2", target_bir_lowering=False, debug=False)

    xT_d = nc.dram_tensor("xT", [C, T], MM_DT, kind="ExternalInput")
    wqT_d = nc.dram_tensor("wqT", [C, HD], MM_DT, kind="ExternalInput")
    wkT_d = nc.dram_tensor("wkT", [C, HD], MM_DT, kind="ExternalInput")
    wvT_d = nc.dram_tensor("wvT", [C, HD], MM_DT, kind="ExternalInput")
    wpT_d = nc.dram_tensor("wpT", [HD, C], MM_DT, kind="ExternalInput")
    mask_d = nc.dram_tensor("mask", [SB, SB], ATT_DT, kind="ExternalInput")
    ones_d = nc.dram_tensor("ones", [SB, NSB * HL], ATT_DT, kind="ExternalInput")
    y_d = nc.dram_tensor("y", [T, C], F32, kind="ExternalOutput")

    NKC = C // SB  # 8 contraction chunks of 128

    with tile.TileContext(nc) as tc:
        with (
            tc.tile_pool(name="persist", bufs=1) as persist,
            tc.tile_pool(name="pt", bufs=5) as pt_pool,
            tc.tile_pool(name="ysb", bufs=3) as ysb_pool,
            tc.tile_pool(name="xtj", bufs=2) as xtj_pool,
            tc.tile_pool(name="norm", bufs=2) as norm_pool,
            tc.tile_pool(name="ps_s", bufs=2, space="PSUM") as ps_s,
            tc.tile_pool(name="ps_o", bufs=4, space="PSUM") as ps_o,
        ):
            ps_mm = ps_s
            ps_y = ps_s
            # ---- persistent SBUF tiles (packed [128, nchunks*width]) ----
            wq_sb = persist.tile([SB, NKC * HD], MM_DT)
            wk_sb = persist.tile([SB, NKC * HD], MM_DT)
            wv_sb = persist.tile([SB, NKC * HD], MM_DT)
            wp_sb = persist.tile([SB, (HD // SB) * C], MM_DT)  # 2 chunks of [128, 1024]
            qT_sb = persist.tile([SB, 2 * T], ATT_DT)  # grp g: heads 2g,2g+1
            kTp_sb = persist.tile([SB, HL * T], ATT_DT)  # head h: cols h*T, rows 64*(h%2), rest zero
            v_sb = persist.tile([SB, NSB * HL * SB], ATT_DT)  # (chunk n, head h): cols (n*HL+h)*128
            oT_sb = persist.tile([SB, 2 * T], MM_DT)
            mask_sb = persist.tile([SB, SB], ATT_DT)

            def load_packed(sb, dram_ap, width):
                n = dram_ap.shape[0] // SB
                nc.sync.dma_start(
                    sb[:].rearrange("p (n w) -> p n w", n=n),
                    dram_ap.rearrange("(n p) w -> p n w", p=SB),
                )

            load_packed(wq_sb, wqT_d.ap(), HD)
            load_packed(wk_sb, wkT_d.ap(), HD)
            load_packed(wv_sb, wvT_d.ap(), HD)
            load_packed(wp_sb, wpT_d.ap(), C)
            nc.sync.dma_start(mask_sb[:], mask_d.ap())

            nc.vector.memset(kTp_sb[:], 0.0)
            nc.vector.memset(v_sb[:], 0.0)

            # ones columns of v_aug (col 64 of each head's 65-col group)
            v_ones = v_sb[:].rearrange("p (k d) -> p k d", k=NSB * HL)[:, :, D : D + 1]  # d=128 now
            nc.sync.dma_start(v_ones, ones_d.ap().unsqueeze(2))

            # ---- phase 2: QKV projections (xT streamed per t-slice j) ----
            xT_packed = xT_d.ap().rearrange("(n p) t -> p n t", p=SB)
            for j in range(NTC):
                xTj = xtj_pool.tile([SB, NKC * TC], MM_DT, tag="xtj", name=f"xtj_{j}")
                nc.sync.dma_start(
                    xTj[:].rearrange("p (n w) -> p n w", n=NKC),
                    xT_packed[:, :, j * TC : (j + 1) * TC],
                )
                for which, w_sb in (("q", wq_sb), ("k", wk_sb)):
                    for g in range(2):  # partition group (2 heads each)
                        ps = ps_mm.tile([SB, TC], F32, tag="s")
                        for n in range(NKC):
                            nc.tensor.matmul(
                                ps[:],
                                _mm(w_sb[:, n * HD + g * SB : n * HD + (g + 1) * SB]),
                                _mm(xTj[:, n * TC : (n + 1) * TC]),
                                start=(n == 0),
                                stop=(n == NKC - 1),
                            )
                        if which == "q":
                            nc.scalar.copy(
                                qT_sb[:, g * T + j * TC : g * T + (j + 1) * TC], ps[:]
                            )
                        else:
                            # head 2g -> rows 0:64, head 2g+1 -> rows 64:128
                            for hh in range(2):
                                h = 2 * g + hh
                                nc.vector.tensor_copy(
                                    kTp_sb[
                                        hh * D : (hh + 1) * D,
                                        h * T + j * TC : h * T + (j + 1) * TC,
                                    ],
                                    ps[hh * D : (hh + 1) * D, :],
                                )

                for n in range(HL * j, HL * j + HL):  # v: out [128 s, 256 d]
                    off = (n - HL * j) * SB
                    ps = ps_mm.tile([SB, TC], F32, tag="s")
                    for m in range(NKC):
                        nc.tensor.matmul(
                            ps[:, 0:HD],
                            _mm(xTj[:, m * TC + off : m * TC + off + SB]),
                            _mm(wv_sb[:, m * HD : (m + 1) * HD]),
                            start=(m == 0),
                            stop=(m == NKC - 1),
                        )
                    dst = v_sb[:, n * HL * SB : (n + 1) * HL * SB].rearrange(
                        "p (h d) -> p h d", h=HL
                    )[:, :, 0:D]
                    src = ps[:, 0:HD].rearrange("p (h d) -> p h d", h=HL)
                    nc.scalar.copy(dst, src)

            # ---- phase 3: attention, 2 head-waves of 2 per t-chunk ----
            for j in range(NTC):
                last_kb = HL * j + 3
                for w in range(2):  # wave = head pair (2w, 2w+1) = partition group w
                    po = [
                        ps_o.tile([SB, TC], F32, tag="o", name=f"po_{j}_{w}_{hh}")
                        for hh in range(2)
                    ]
                    for kb in range(last_kb + 1):
                        tstart = max(0, (kb - HL * j) * SB)
                        nn = TC - tstart
                        pss = ps_s.tile(
                            [SB, 2 * TC], F32, tag="s", name=f"pss_{j}_{w}_{kb}"
                        )
                        pT = pt_pool.tile(
                            [SB, 2 * TC], ATT_DT, tag="pt", name=f"pt_{j}_{w}_{kb}"
                        )
                        for hh in range(2):
                            h = 2 * w + hh
                            nc.tensor.matmul(
                                pss[:, hh * TC + tstart : (hh + 1) * TC],
                                _mm(kTp_sb[:, h * T + kb * SB : h * T + (kb + 1) * SB]),
                                _mm(
                                    qT_sb[
                                        :,
                                        w * T + j * TC + tstart : w * T + (j + 1) * TC,
                                    ]
                                ),
                                start=True,
                                stop=True,
                            )
                        pss3 = pss[:].rearrange("p (h t) -> p h t", h=2)
                        pT3 = pT[:].rearrange("p (h t) -> p h t", h=2)
                        nc.scalar.activation(
                            pT3[:, :, tstart:],
                            pss3[:, :, tstart:],
                            mybir.ActivationFunctionType.Exp,
                            scale=float(D) ** -0.5,
                        )
                        if kb >= HL * j:  # diagonal block: causal tri-mask
                            nc.vector.tensor_mul(
                                pT3[:, :, tstart : tstart + SB],
                                pT3[:, :, tstart : tstart + SB],
                                mask_sb[:].unsqueeze(1).to_broadcast((SB, 2, SB)),
                            )
                        for hh in range(2):
                            h = 2 * w + hh
                            nc.tensor.matmul(
                                po[hh][:, tstart:],
                                _mm(
                                    v_sb[:, (kb * HL + h) * SB : (kb * HL + h + 1) * SB]
                                ),
                                _mm(pT[:, hh * TC + tstart : (hh + 1) * TC]),
                                start=(kb == 0),
                                stop=(kb == last_kb),
                            )
                    # normalize: oT_sb[...] = po[0:64] * (1 / po[64])
                    for hh in range(2):
                        h = 2 * w + hh
                        hp = D * (h % 2)
                        tmp = norm_pool.tile(
                            [D, TC], F32, tag="tmp", name=f"tmp_{j}_{h}"
                        )
                        nc.scalar.copy(tmp[:], po[hh][0:D, :])
                        row = norm_pool.tile(
                            [1, TC], F32, tag="row", name=f"row_{j}_{h}"
                        )
                        nc.scalar.copy(row[:], po[hh][D : D + 1, :])  # releases po
                        bc = norm_pool.tile([D, TC], F32, tag="bc", name=f"bc_{j}_{h}")
                        nc.gpsimd.partition_broadcast(bc[:], row[:])
                        nc.vector.reciprocal(bc[:], bc[:])
                        nc.vector.tensor_mul(
                            oT_sb[hp : hp + D, w * T + j * TC : w * T + (j + 1) * TC],
                            tmp[:],
                            bc[:],
                        )

            # ---- phase 4: output projection ----
            for i in range(NSB):
                for half in range(2):
                    ps = ps_y.tile([SB, TC], F32, tag="s")
                    for g in range(2):
                        nc.tensor.matmul(
                            ps[:],
                            _mm(oT_sb[:, g * T + i * SB : g * T + (i + 1) * SB]),
                            _mm(wp_sb[:, g * C + half * TC : g * C + (half + 1) * TC]),
                            start=(g == 0),
                            stop=(g == 1),
                        )
                    y_sb = ysb_pool.tile([SB, TC], F32, tag="ysb")
                    nc.vector.tensor_copy(y_sb[:], ps[:])
                    nc.sync.dma_start(
                        y_d.ap()[i * SB : (i + 1) * SB, half * TC : (half + 1) * TC],
                        y_sb[:],
                    )

    nc.compile()
    return nc


_NC_CACHE = None


def _get_program():
    global _NC_CACHE
    if _NC_CACHE is None:
        _NC_CACHE = _build_program()
    return _NC_CACHE


def _make_in_maps(x, W_k, W_q, W_v, W_proj):
    import ml_dtypes

    att_np = ml_dtypes.bfloat16 if ATT_DT == BF16 else np.float32
    mask = np.triu(np.ones((SB, SB), dtype=att_np))  # mask[s,t]=1 iff s<=t
    in_maps = []
    for c in range(N_CORES):
        b, hg = c // 4, c % 4
        rows = slice(hg * HD, (hg + 1) * HD)
        in_maps.append(
            {
                "xT": np.ascontiguousarray(x[b].T).astype(np.float32),
                "wqT": np.ascontiguousarray(W_q[rows].T).astype(np.float32),
                "wkT": np.ascontiguousarray(W_k[rows].T).astype(np.float32),
                "wvT": np.ascontiguousarray(W_v[rows].T).astype(np.float32),
                "wpT": np.ascontiguousarray(W_proj[:, rows].T).astype(np.float32),
                "mask": mask,
                "ones": np.ones((SB, NSB * HL), dtype=att_np),
            }
        )
    return in_maps


def _run(x, W_k, W_q, W_v, W_proj, **spmd_kwargs):
    nc = _get_program()
    in_maps = _make_in_maps(x, W_k, W_q, W_v, W_proj)
    res = run_bass_kernel_spmd(nc, in_maps, list(range(N_CORES)), **spmd_kwargs)
    ys = [res.results[c]["y"] for c in range(N_CORES)]
    out = np.stack(
        [
            ys[0] + ys[1] + ys[2] + ys[3],
            ys[4] + ys[5] + ys[6] + ys[7],
        ]
    ).astype(np.float32)
    return out, res


def kernel(x, W_k, W_q, W_v, W_proj):
    out, _ = _run(
        np.asarray(x), np.asarray(W_k), np.asarray(W_q), np.asarray(W_v),
        np.asarray(W_proj),
    )
    return out


# revision 18
# speedup vs baseline: 1.5835x; 1.0071x over previous
"""Causal multi-head attention (B=2, T=2048, C=1024, H=16, D=64) on 8 TRN2 cores.

Sharding: core c -> batch b = c//4, head-group hg = c%4 (4 heads/core).
Each core computes its 4 heads' attention and a partial output projection
(contraction over its 256 feature columns of W_proj); the host sums the 4
partials per batch.

All device-side layouts are transposed on host so the kernel needs no
on-device transposes:
  xT  [C, T]   = x[b].T
  wqT/wkT/wvT [C, 256] = W_{q,k,v}[rows].T
  wpT [256, C] = W_proj[:, rows].T
Attention math per head (D=64):
  qT/kT [64, T] = (wqT chunk).T @ xT            (PE, f32r)
  v     [T, 64] = (xT chunk).T @ wvT            (+ ones col -> row sums)
  sT    [s, t]  = kT.T @ qT                     (K=64)
  pT            = exp(sT/8)  (no max-subtraction needed: |scores/8| < ~7)
  causal: tri-mask on diagonal 128-blocks only; lower kb blocks restricted
  oT_aug[65, t] = v_aug.T @ pT    (row 64 = softmax sums, free)
  oT_norm       = oT * (1/sums)   (reciprocal + DMA partition-broadcast + mul)
  y     [t, p]  = oT.T @ wpT      (partial over this core's 256 features)
"""

import sys

sys.path.insert(0, "/opt/trn_rl_repo")

import numpy as np

import concourse.bass as bass  # noqa: E402
import concourse.mybir as mybir  # noqa: E402
import concourse.tile as tile  # noqa: E402
from concourse import bacc  # noqa: E402
from concourse.bass_utils import run_bass_kernel_spmd  # noqa: E402

F32 = mybir.dt.float32
F32R = mybir.dt.float32r

T = 2048
C = 1024
HL = 4  # heads per core
D = 64
HD = HL * D  # 256 local feature dim
TC = 512  # t-chunk for attention
NTC = T // TC  # 4
SB = 128  # s block
NSB = T // SB  # 16
N_CORES = 8

# matmul input dtype: float32r streams 1 row/cycle at N>=256 (4x faster than
# plain float32) with near-fp32 accuracy (fp32 accumulate).
MM_DT = F32R
BF16 = mybir.dt.bfloat16
# attention (scores/PV) operand dtype
ATT_DT = BF16


def _mm(ap):
    return ap


def _build_program():
    nc = bacc.Bacc("TRN2", target_bir_lowering=False, debug=False)

    xT_d = nc.dram_tensor("xT", [C, T], MM_DT, kind="ExternalInput")
    wqT_d = nc.dram_tensor("wqT", [C, HD], MM_DT, kind="ExternalInput")
    wkT_d = nc.dram_tensor("wkT", [C, HD], MM_DT, kind="ExternalInput")
    wvT_d = nc.dram_tensor("wvT", [C, HD], MM_DT, kind="ExternalInput")
    wpT_d = nc.dram_tensor("wpT", [HD, C], MM_DT, kind="ExternalInput")
    mask_d = nc.dram_tensor("mask", [SB, SB], ATT_DT, kind="ExternalInput")
    ones_d = nc.dram_tensor("ones", [SB, NSB * HL], ATT_DT, kind="ExternalInput")
    y_d = nc.dram_tensor("y", [T, C], F32, kind="ExternalOutput")

    NKC = C // SB  # 8 contraction chunks of 128

    with tile.TileContext(nc) as tc:
        with (
            tc.tile_pool(name="persist", bufs=1) as persist,
            tc.tile_pool(name="pt", bufs=6) as pt_pool,
            tc.tile_pool(name="ysb", bufs=3) as ysb_pool,
            tc.tile_pool(name="xtj", bufs=2) as xtj_pool,
            tc.tile_pool(name="norm", bufs=2) as norm_pool,
            tc.tile_pool(name="ps_s", bufs=2, space="PSUM") as ps_s,
            tc.tile_pool(name="ps_o", bufs=4, space="PSUM") as ps_o,
        ):
            ps_mm = ps_s
            ps_y = ps_s
            # ---- persistent SBUF tiles (packed [128, nchunks*width]) ----
            wq_sb = persist.tile([SB, NKC * HD], MM_DT)
            wk_sb = persist.tile([SB, NKC * HD], MM_DT)
            wv_sb = persist.tile([SB, NKC * HD], MM_DT)
            wp_sb = persist.tile([SB, (HD // SB) * C], MM_DT)  # 2 chunks of [128, 1024]
            qT_sb = persist.tile([SB, 2 * T], ATT_DT)  # grp g: heads 2g,2g+1
            kTp_sb = persist.tile([SB, HL * T], ATT_DT)  # head h: cols h*T, rows 64*(h%2), rest zero
            v_sb = persist.tile([SB, NSB * HL * SB], ATT_DT)  # (chunk n, head h): cols (n*HL+h)*128
            oT_sb = persist.tile([SB, 2 * T], MM_DT)
            mask_sb = persist.tile([SB, SB], ATT_DT)

            def load_packed(sb, dram_ap, width):
                n = dram_ap.shape[0] // SB
                nc.sync.dma_start(
                    sb[:].rearrange("p (n w) -> p n w", n=n),
                    dram_ap.rearrange("(n p) w -> p n w", p=SB),
                )

            load_packed(wq_sb, wqT_d.ap(), HD)
            load_packed(wk_sb, wkT_d.ap(), HD)
            load_packed(wv_sb, wvT_d.ap(), HD)
            nc.sync.dma_start(mask_sb[:], mask_d.ap())

            nc.vector.memset(kTp_sb[:], 0.0)
            nc.vector.memset(v_sb[:], 0.0)

            # ones columns of v_aug (col 64 of each head's 65-col group)
            v_ones = v_sb[:].rearrange("p (k d) -> p k d", k=NSB * HL)[:, :, D : D + 1]  # d=128 now
            nc.sync.dma_start(v_ones, ones_d.ap().unsqueeze(2))

            # ---- phase 2: QKV projections (xT streamed per t-slice j) ----
            xT_packed = xT_d.ap().rearrange("(n p) t -> p n t", p=SB)
            for j in range(NTC):
                xTj = xtj_pool.tile([SB, NKC * TC], MM_DT, tag="xtj", name=f"xtj_{j}")
                nc.sync.dma_start(
                    xTj[:].rearrange("p (n w) -> p n w", n=NKC),
                    xT_packed[:, :, j * TC : (j + 1) * TC],
                )
                for which, w_sb in (("q", wq_sb), ("k", wk_sb)):
                    for g in range(2):  # partition group (2 heads each)
                        ps = ps_mm.tile([SB, TC], F32, tag="s")
                        for n in range(NKC):
                            nc.tensor.matmul(
                                ps[:],
                                _mm(w_sb[:, n * HD + g * SB : n * HD + (g + 1) * SB]),
                                _mm(xTj[:, n * TC : (n + 1) * TC]),
                                start=(n == 0),
                                stop=(n == NKC - 1),
                            )
                        if which == "q":
                            nc.vector.tensor_copy(
                                qT_sb[:, g * T + j * TC : g * T + (j + 1) * TC], ps[:]
                            )
                        else:
                            # head 2g -> rows 0:64, head 2g+1 -> rows 64:128
                            for hh in range(2):
                                h = 2 * g + hh
                                nc.vector.tensor_copy(
                                    kTp_sb[
                                        hh * D : (hh + 1) * D,
                                        h * T + j * TC : h * T + (j + 1) * TC,
                                    ],
                                    ps[hh * D : (hh + 1) * D, :],
                                )

                for n in range(HL * j, HL * j + HL):  # v: out [128 s, 256 d]
                    off = (n - HL * j) * SB
                    ps = ps_mm.tile([SB, TC], F32, tag="s")
                    for m in range(NKC):
                        nc.tensor.matmul(
                            ps[:, 0:HD],
                            _mm(xTj[:, m * TC + off : m * TC + off + SB]),
                            _mm(wv_sb[:, m * HD : (m + 1) * HD]),
                            start=(m == 0),
                            stop=(m == NKC - 1),
                        )
                    dst = v_sb[:, n * HL * SB : (n + 1) * HL * SB].rearrange(
                        "p (h d) -> p h d", h=HL
                    )[:, :, 0:D]
                    src = ps[:, 0:HD].rearrange("p (h d) -> p h d", h=HL)
                    nc.scalar.copy(dst, src)

            # ---- phase 3: attention, 2 head-waves of 2 per t-chunk ----
            load_packed(wp_sb, wpT_d.ap(), C)
            for j in range(NTC):
                last_kb = HL * j + 3
                for w in range(2):  # wave = head pair (2w, 2w+1) = partition group w
                    po = [
                        ps_o.tile([SB, TC], F32, tag="o", name=f"po_{j}_{w}_{hh}")
                        for hh in range(2)
                    ]
                    for kb in range(last_kb + 1):
                        tstart = max(0, (kb - HL * j) * SB)
                        nn = TC - tstart
                        pss = ps_s.tile(
                            [SB, 2 * TC], F32, tag="s", name=f"pss_{j}_{w}_{kb}"
                        )
                        pT = pt_pool.tile(
                            [SB, 2 * TC], ATT_DT, tag="pt", name=f"pt_{j}_{w}_{kb}"
                        )
                        for hh in range(2):
                            h = 2 * w + hh
                            nc.tensor.matmul(
                                pss[:, hh * TC + tstart : (hh + 1) * TC],
                                _mm(kTp_sb[:, h * T + kb * SB : h * T + (kb + 1) * SB]),
                                _mm(
                                    qT_sb[
                                        :,
                                        w * T + j * TC + tstart : w * T + (j + 1) * TC,
                                    ]
                                ),
                                start=True,
                                stop=True,
                            )
                        pss3 = pss[:].rearrange("p (h t) -> p h t", h=2)
                        pT3 = pT[:].rearrange("p (h t) -> p h t", h=2)
                        nc.scalar.activation(
                            pT3[:, :, tstart:],
                            pss3[:, :, tstart:],
                            mybir.ActivationFunctionType.Exp,
                            scale=float(D) ** -0.5,
                        )
                        if kb >= HL * j:  # diagonal block: causal tri-mask
                            nc.vector.tensor_mul(
                                pT3[:, :, tstart : tstart + SB],
                                pT3[:, :, tstart : tstart + SB],
                                mask_sb[:].unsqueeze(1).to_broadcast((SB, 2, SB)),
                            )
                        for hh in range(2):
                            h = 2 * w + hh
                            nc.tensor.matmul(
                                po[hh][:, tstart:],
                                _mm(
                                    v_sb[:, (kb * HL + h) * SB : (kb * HL + h + 1) * SB]
                                ),
                                _mm(pT[:, hh * TC + tstart : (hh + 1) * TC]),
                                start=(kb == 0),
                                stop=(kb == last_kb),
                            )
                    # normalize: oT_sb[...] = po[0:64] * (1 / po[64])
                    for hh in range(2):
                        h = 2 * w + hh
                        hp = D * (h % 2)
                        tmp = norm_pool.tile(
                            [D, TC], F32, tag="tmp", name=f"tmp_{j}_{h}"
                        )
                        nc.scalar.copy(tmp[:], po[hh][0:D, :])
                        row = norm_pool.tile(
                            [1, TC], F32, tag="row", name=f"row_{j}_{h}"
                        )
                        nc.scalar.copy(row[:], po[hh][D : D + 1, :])  # releases po
                        bc = norm_pool.tile([D, TC], F32, tag="bc", name=f"bc_{j}_{h}")
                        nc.gpsimd.partition_broadcast(bc[:], row[:])
                        rec = norm_pool.tile([D, TC], F32, tag="rec", name=f"rec_{j}_{h}")
                        scr = norm_pool.tile([D, TC], F32, tag="scr", name=f"scr_{j}_{h}")
                        nc.vector.reciprocal_approx_accurate(rec[:], bc[:], scr[:])
                        nc.vector.tensor_mul(
                            oT_sb[hp : hp + D, w * T + j * TC : w * T + (j + 1) * TC],
                            tmp[:],
                            rec[:],
                        )

                # ---- output projection for this t-chunk ----
                for i in range(HL * j, HL * j + HL):
                    for half in range(2):
                        ps = ps_y.tile([SB, TC], F32, tag="s", name=f"psy_{i}_{half}")
                        for g in range(2):
                            nc.tensor.matmul(
                                ps[:],
                                _mm(oT_sb[:, g * T + i * SB : g * T + (i + 1) * SB]),
                                _mm(
                                    wp_sb[
                                        :, g * C + half * TC : g * C + (half + 1) * TC
                                    ]
                                ),
                                start=(g == 0),
                                stop=(g == 1),
                            )
                        y_sb = ysb_pool.tile([SB, TC], F32, tag="ysb")
                        nc.vector.tensor_copy(y_sb[:], ps[:])
                        nc.sync.dma_start(
                            y_d.ap()[
                                i * SB : (i + 1) * SB, half * TC : (half + 1) * TC
                            ],
                            y_sb[:],
                        )

    nc.compile()
    return nc


_NC_CACHE = None


def _get_program():
    global _NC_CACHE
    if _NC_CACHE is None:
        _NC_CACHE = _build_program()
    return _NC_CACHE


def _make_in_maps(x, W_k, W_q, W_v, W_proj):
    import ml_dtypes

    att_np = ml_dtypes.bfloat16 if ATT_DT == BF16 else np.float32
    mask = np.triu(np.ones((SB, SB), dtype=att_np))  # mask[s,t]=1 iff s<=t
    in_maps = []
    for c in range(N_CORES):
        b, hg = c // 4, c % 4
        rows = slice(hg * HD, (hg + 1) * HD)
        in_maps.append(
            {
                "xT": np.ascontiguousarray(x[b].T).astype(np.float32),
                "wqT": np.ascontiguousarray(W_q[rows].T).astype(np.float32),
                "wkT": np.ascontiguousarray(W_k[rows].T).astype(np.float32),
                "wvT": np.ascontiguousarray(W_v[rows].T).astype(np.float32),
                "wpT": np.ascontiguousarray(W_proj[:, rows].T).astype(np.float32),
                "mask": mask,
                "ones": np.ones((SB, NSB * HL), dtype=att_np),
            }
        )
    return in_maps


def _run(x, W_k, W_q, W_v, W_proj, **spmd_kwargs):
    nc = _get_program()
    in_maps = _make_in_maps(x, W_k, W_q, W_v, W_proj)
    res = run_bass_kernel_spmd(nc, in_maps, list(range(N_CORES)), **spmd_kwargs)
    ys = [res.results[c]["y"] for c in range(N_CORES)]
    out = np.stack(
        [
            ys[0] + ys[1] + ys[2] + ys[3],
            ys[4] + ys[5] + ys[6] + ys[7],
        ]
    ).astype(np.float32)
    return out, res


def kernel(x, W_k, W_q, W_v, W_proj):
    out, _ = _run(
        np.asarray(x), np.asarray(W_k), np.asarray(W_q), np.asarray(W_v),
        np.asarray(W_proj),
    )
    return out


# revision 19
# speedup vs baseline: 1.8667x; 1.1788x over previous
"""Causal multi-head attention (B=2, T=2048, C=1024, H=16, D=64) on 8 TRN2 cores.

Sharding: core c -> batch b = c//4, head-group hg = c%4 (4 heads/core).
Each core computes its 4 heads' attention and a partial output projection
(contraction over its 256 feature columns of W_proj); the host sums the 4
partials per batch.

All device-side layouts are transposed on host so the kernel needs no
on-device transposes:
  xT  [C, T]   = x[b].T
  wqT/wkT/wvT [C, 256] = W_{q,k,v}[rows].T
  wpT [256, C] = W_proj[:, rows].T
Attention math per head (D=64):
  qT/kT [64, T] = (wqT chunk).T @ xT            (PE, f32r)
  v     [T, 64] = (xT chunk).T @ wvT            (+ ones col -> row sums)
  sT    [s, t]  = kT.T @ qT                     (K=64)
  pT            = exp(sT/8)  (no max-subtraction needed: |scores/8| < ~7)
  causal: tri-mask on diagonal 128-blocks only; lower kb blocks restricted
  oT_aug[65, t] = v_aug.T @ pT    (row 64 = softmax sums, free)
  oT_norm       = oT * (1/sums)   (reciprocal + DMA partition-broadcast + mul)
  y     [t, p]  = oT.T @ wpT      (partial over this core's 256 features)
"""

import sys

sys.path.insert(0, "/opt/trn_rl_repo")

import numpy as np

import concourse.bass as bass  # noqa: E402
import concourse.mybir as mybir  # noqa: E402
import concourse.tile as tile  # noqa: E402
from concourse import bacc  # noqa: E402
from concourse.bass_utils import run_bass_kernel_spmd  # noqa: E402

F32 = mybir.dt.float32
F32R = mybir.dt.float32r

T = 2048
C = 1024
HL = 4  # heads per core
D = 64
HD = HL * D  # 256 local feature dim
TC = 512  # t-chunk for attention
NTC = T // TC  # 4
SB = 128  # s block
NSB = T // SB  # 16
N_CORES = 8

# matmul input dtype: float32r streams 1 row/cycle at N>=256 (4x faster than
# plain float32) with near-fp32 accuracy (fp32 accumulate).
MM_DT = F32R
BF16 = mybir.dt.bfloat16
# attention (scores/PV) operand dtype
ATT_DT = BF16


def _mm(ap):
    return ap


def _build_program():
    nc = bacc.Bacc("TRN2", target_bir_lowering=False, debug=False)

    xT_d = nc.dram_tensor("xT", [C, T], MM_DT, kind="ExternalInput")
    wqT_d = nc.dram_tensor("wqT", [C, HD], MM_DT, kind="ExternalInput")
    wkT_d = nc.dram_tensor("wkT", [C, HD], MM_DT, kind="ExternalInput")
    wvT_d = nc.dram_tensor("wvT", [C, HD], MM_DT, kind="ExternalInput")
    wpT_d = nc.dram_tensor("wpT", [HD, C], MM_DT, kind="ExternalInput")
    mask_d = nc.dram_tensor("mask", [SB, SB], ATT_DT, kind="ExternalInput")
    ones_d = nc.dram_tensor("ones", [SB, NSB * HL], ATT_DT, kind="ExternalInput")
    y_d = nc.dram_tensor("y", [T, C], F32, kind="ExternalOutput")

    NKC = C // SB  # 8 contraction chunks of 128

    with tile.TileContext(nc) as tc:
        with (
            tc.tile_pool(name="persist", bufs=1) as persist,
            tc.tile_pool(name="pt", bufs=6) as pt_pool,
            tc.tile_pool(name="ysb", bufs=3) as ysb_pool,
            tc.tile_pool(name="xtj", bufs=2) as xtj_pool,
            tc.tile_pool(name="norm", bufs=2) as norm_pool,
            tc.tile_pool(name="ps_s", bufs=2, space="PSUM") as ps_s,
            tc.tile_pool(name="ps_o", bufs=4, space="PSUM") as ps_o,
        ):
            ps_mm = ps_s
            ps_y = ps_o
            # ---- persistent SBUF tiles (packed [128, nchunks*width]) ----
            wq_sb = persist.tile([SB, NKC * HD], MM_DT)
            wk_sb = persist.tile([SB, NKC * HD], MM_DT)
            wv_sb = persist.tile([SB, NKC * HD], MM_DT)
            wp_sb = persist.tile([SB, (HD // SB) * C], MM_DT)  # 2 chunks of [128, 1024]
            qT_sb = persist.tile([SB, 2 * T], ATT_DT)  # grp g: heads 2g,2g+1
            kTp_sb = persist.tile([SB, HL * T], ATT_DT)  # head h: cols h*T, rows 64*(h%2), rest zero
            v_sb = persist.tile([SB, NSB * HL * SB], ATT_DT)  # (chunk n, head h): cols (n*HL+h)*128
            oT_sb = persist.tile([SB, 2 * T], MM_DT)
            mask_sb = persist.tile([SB, SB], ATT_DT)

            def load_packed(sb, dram_ap, width):
                n = dram_ap.shape[0] // SB
                nc.sync.dma_start(
                    sb[:].rearrange("p (n w) -> p n w", n=n),
                    dram_ap.rearrange("(n p) w -> p n w", p=SB),
                )

            xT_packed = xT_d.ap().rearrange("(n p) t -> p n t", p=SB)
            xTj_tiles = {}
            xTj_tiles[0] = xtj_pool.tile(
                [SB, NKC * TC], MM_DT, tag="xtj", name="xtj_0"
            )
            nc.sync.dma_start(
                xTj_tiles[0][:].rearrange("p (n w) -> p n w", n=NKC),
                xT_packed[:, :, 0:TC],
            )
            load_packed(wq_sb, wqT_d.ap(), HD)
            load_packed(wk_sb, wkT_d.ap(), HD)
            load_packed(wv_sb, wvT_d.ap(), HD)
            nc.sync.dma_start(mask_sb[:], mask_d.ap())

            nc.vector.memset(kTp_sb[:], 0.0)
            nc.vector.memset(v_sb[:], 0.0)

            # ones columns of v_aug (col 64 of each head's 65-col group)
            v_ones = v_sb[:].rearrange("p (k d) -> p k d", k=NSB * HL)[:, :, D : D + 1]  # d=128 now
            nc.sync.dma_start(v_ones, ones_d.ap().unsqueeze(2))

            # ---- phase 2: QKV projections (xT streamed per t-slice j) ----
            for j in range(NTC):
                if j in xTj_tiles:
                    xTj = xTj_tiles[j]
                else:
                    xTj = xtj_pool.tile(
                        [SB, NKC * TC], MM_DT, tag="xtj", name=f"xtj_{j}"
                    )
                    nc.sync.dma_start(
                        xTj[:].rearrange("p (n w) -> p n w", n=NKC),
                        xT_packed[:, :, j * TC : (j + 1) * TC],
                    )
                for which, w_sb in (("q", wq_sb), ("k", wk_sb)):
                    for g in range(2):  # partition group (2 heads each)
                        ps = ps_mm.tile([SB, TC], F32, tag="s")
                        for n in range(NKC):
                            nc.tensor.matmul(
                                ps[:],
                                _mm(w_sb[:, n * HD + g * SB : n * HD + (g + 1) * SB]),
                                _mm(xTj[:, n * TC : (n + 1) * TC]),
                                start=(n == 0),
                                stop=(n == NKC - 1),
                            )
                        if which == "q":
                            nc.vector.tensor_copy(
                                qT_sb[:, g * T + j * TC : g * T + (j + 1) * TC], ps[:]
                            )
                        else:
                            # head 2g -> rows 0:64, head 2g+1 -> rows 64:128
                            for hh in range(2):
                                h = 2 * g + hh
                                nc.vector.tensor_copy(
                                    kTp_sb[
                                        hh * D : (hh + 1) * D,
                                        h * T + j * TC : h * T + (j + 1) * TC,
                                    ],
                                    ps[hh * D : (hh + 1) * D, :],
                                )

                for n in range(HL * j, HL * j + HL):  # v: out [128 s, 256 d]
                    off = (n - HL * j) * SB
                    ps = ps_mm.tile([SB, TC], F32, tag="s")
                    for m in range(NKC):
                        nc.tensor.matmul(
                            ps[:, 0:HD],
                            _mm(xTj[:, m * TC + off : m * TC + off + SB]),
                            _mm(wv_sb[:, m * HD : (m + 1) * HD]),
                            start=(m == 0),
                            stop=(m == NKC - 1),
                        )
                    dst = v_sb[:, n * HL * SB : (n + 1) * HL * SB].rearrange(
                        "p (h d) -> p h d", h=HL
                    )[:, :, 0:D]
                    src = ps[:, 0:HD].rearrange("p (h d) -> p h d", h=HL)
                    nc.scalar.copy(dst, src)

            # ---- phase 3: attention, 2 head-waves of 2 per t-chunk ----
            load_packed(wp_sb, wpT_d.ap(), C)
            for j in range(NTC):
                last_kb = HL * j + 3
                for w in range(2):  # wave = head pair (2w, 2w+1) = partition group w
                    po = [
                        ps_o.tile([SB, TC], F32, tag="o", name=f"po_{j}_{w}_{hh}")
                        for hh in range(2)
                    ]
                    for kb in range(last_kb + 1):
                        tstart = max(0, (kb - HL * j) * SB)
                        nn = TC - tstart
                        pss = ps_s.tile(
                            [SB, 2 * TC], F32, tag="s", name=f"pss_{j}_{w}_{kb}"
                        )
                        pT = pt_pool.tile(
                            [SB, 2 * TC], ATT_DT, tag="pt", name=f"pt_{j}_{w}_{kb}"
                        )
                        for hh in range(2):
                            h = 2 * w + hh
                            nc.tensor.matmul(
                                pss[:, hh * TC + tstart : (hh + 1) * TC],
                                _mm(kTp_sb[:, h * T + kb * SB : h * T + (kb + 1) * SB]),
                                _mm(
                                    qT_sb[
                                        :,
                                        w * T + j * TC + tstart : w * T + (j + 1) * TC,
                                    ]
                                ),
                                start=True,
                                stop=True,
                            )
                        pss3 = pss[:].rearrange("p (h t) -> p h t", h=2)
                        pT3 = pT[:].rearrange("p (h t) -> p h t", h=2)
                        nc.scalar.activation(
                            pT3[:, :, tstart:],
                            pss3[:, :, tstart:],
                            mybir.ActivationFunctionType.Exp,
                            scale=float(D) ** -0.5,
                        )
                        if kb >= HL * j:  # diagonal block: causal tri-mask
                            nc.vector.tensor_mul(
                                pT3[:, :, tstart : tstart + SB],
                                pT3[:, :, tstart : tstart + SB],
                                mask_sb[:].unsqueeze(1).to_broadcast((SB, 2, SB)),
                            )
                        for hh in range(2):
                            h = 2 * w + hh
                            nc.tensor.matmul(
                                po[hh][:, tstart:],
                                _mm(
                                    v_sb[:, (kb * HL + h) * SB : (kb * HL + h + 1) * SB]
                                ),
                                _mm(pT[:, hh * TC + tstart : (hh + 1) * TC]),
                                start=(kb == 0),
                                stop=(kb == last_kb),
                            )
                    # normalize: oT_sb[...] = po[0:64] * (1 / po[64])
                    for hh in range(2):
                        h = 2 * w + hh
                        hp = D * (h % 2)
                        tmp = norm_pool.tile(
                            [D, TC], F32, tag="tmp", name=f"tmp_{j}_{h}"
                        )
                        nc.vector.tensor_copy(tmp[:], po[hh][0:D, :])
                        row = norm_pool.tile(
                            [1, TC], F32, tag="row", name=f"row_{j}_{h}"
                        )
                        nc.scalar.copy(row[:], po[hh][D : D + 1, :])  # releases po
                        bc = norm_pool.tile([D, TC], F32, tag="bc", name=f"bc_{j}_{h}")
                        nc.gpsimd.partition_broadcast(bc[:], row[:])
                        rec = norm_pool.tile([D, TC], F32, tag="rec", name=f"rec_{j}_{h}")
                        scr = norm_pool.tile([D, TC], F32, tag="scr", name=f"scr_{j}_{h}")
                        nc.vector.reciprocal_approx_accurate(rec[:], bc[:], scr[:])
                        nc.vector.tensor_mul(
                            oT_sb[hp : hp + D, w * T + j * TC : w * T + (j + 1) * TC],
                            tmp[:],
                            rec[:],
                        )

                # ---- output projection for this t-chunk ----
                for i in range(HL * j, HL * j + HL):
                    for half in range(2):
                        ps = ps_y.tile([SB, TC], F32, tag="o", name=f"psy_{i}_{half}")
                        for g in range(2):
                            nc.tensor.matmul(
                                ps[:],
                                _mm(oT_sb[:, g * T + i * SB : g * T + (i + 1) * SB]),
                                _mm(
                                    wp_sb[
                                        :, g * C + half * TC : g * C + (half + 1) * TC
                                    ]
                                ),
                                start=(g == 0),
                                stop=(g == 1),
                            )
                        y_sb = ysb_pool.tile([SB, TC], F32, tag="ysb")
                        nc.vector.tensor_copy(y_sb[:], ps[:])
                        nc.sync.dma_start(
                            y_d.ap()[
                                i * SB : (i + 1) * SB, half * TC : (half + 1) * TC
                            ],
                            y_sb[:],
                        )

    nc.compile()
    return nc


_NC_CACHE = None


def _get_program():
    global _NC_CACHE
    if _NC_CACHE is None:
        _NC_CACHE = _build_program()
    return _NC_CACHE


def _make_in_maps(x, W_k, W_q, W_v, W_proj):
    import ml_dtypes

    att_np = ml_dtypes.bfloat16 if ATT_DT == BF16 else np.float32
    mask = np.triu(np.ones((SB, SB), dtype=att_np))  # mask[s,t]=1 iff s<=t
    in_maps = []
    for c in range(N_CORES):
        b, hg = c // 4, c % 4
        rows = slice(hg * HD, (hg + 1) * HD)
        in_maps.append(
            {
                "xT": np.ascontiguousarray(x[b].T).astype(np.float32),
                "wqT": np.ascontiguousarray(W_q[rows].T).astype(np.float32),
                "wkT": np.ascontiguousarray(W_k[rows].T).astype(np.float32),
                "wvT": np.ascontiguousarray(W_v[rows].T).astype(np.float32),
                "wpT": np.ascontiguousarray(W_proj[:, rows].T).astype(np.float32),
                "mask": mask,
                "ones": np.ones((SB, NSB * HL), dtype=att_np),
            }
        )
    return in_maps


def _run(x, W_k, W_q, W_v, W_proj, **spmd_kwargs):
    nc = _get_program()
    in_maps = _make_in_maps(x, W_k, W_q, W_v, W_proj)
    res = run_bass_kernel_spmd(nc, in_maps, list(range(N_CORES)), **spmd_kwargs)
    ys = [res.results[c]["y"] for c in range(N_CORES)]
    out = np.stack(
        [
            ys[0] + ys[1] + ys[2] + ys[3],
            ys[4] + ys[5] + ys[6] + ys[7],
        ]
    ).astype(np.float32)
    return out, res


def kernel(x, W_k, W_q, W_v, W_proj):
    out, _ = _run(
        np.asarray(x), np.asarray(W_k), np.asarray(W_q), np.asarray(W_v),
        np.asarray(W_proj),
    )
    return out
